# revision 42
# baseline (speedup 1.0000x reference)
"""Trainium2 Bass kernel for nn_AttentionBlock (B=2, C=256, D=8, H=32, W=32).

reference math:
    xf = x.reshape(B, C, N)                        # N = 8192
    q = wq @ xf + bq                               # (B, 32, N)
    k = wk @ xf + bk                               # (B, 32, N)
    v = wv @ xf + bv                               # (B, 256, N)
    attn = softmax(q^T k, axis=-1)                 # (B, N, N)
    out = attn @ v^T                               # (B, N, C) buffer
    result = gamma * out.reshape(B, C, d, h, w) + x

Sharding (8 cores): core i -> batch b = i//4, query-chunk c = i%4 of 2048
rows.  No collectives.

v6 design (v4 + split exp + host pre-layout + key rotation):
  - host pre-arranges every tensor into its exact SBUF layout so all
    input DMAs are contiguous (fast SWDGE descriptors, early start).
  - keys are rotated per-core on host so each core's own quarter is
    first: q = xf8[:, :, 0:2048] is a free SBUF view (no q DMA), and
    the projection pipeline starts right after the first 0.5MB lands.
  - wq is host-replicated x4 along output channels, so the q-projection
    matmul directly produces the 4x partition-replicated q_rep that the
    block-diagonal score matmuls need (no SBUF replicate DMAs).
  - projections: fp8 weights (prescaled x64), DoubleRow K=256; per
    quarter: 4 k-tiles + 16 v-tiles issued as DMA lands.  q/k bias
    descale on ScalarE (Identity), v-copies on DVE.
  - scores: S^T = k^T q as bf16 K=32 matmuls via the 4-quadrant
    block-diagonal k_bd (dense PE stream at 128 outputs/cycle).
    (tile_position row-group variants measured SLOWER: ~230ns PE
    reconfig penalty on every scores<->attn@v transition.)
  - softmax exp (|S| < 4.5, no max subtraction): each superstep's two
    512-col S banks split: bank0 -> ScalarE (ACTIVATE Exp -> fp8),
    bank1 -> DVE (Schraudolph round(a*S+b) as int8 bitcast fp8e4).
    Both run inside one 855ns superstep so the PE never waits on exp.
  - attn @ v: fp8 DoubleRow, pipe lag 1.  vT is 272-padded and carries
    a WS=64.0 column so PSUM accumulates WS*rowsum next to out.
  - epilogue: rec = 1/psum[:,256]; out = psum[:,0:256]*rec + xres
    (one DVE scalar_tensor_tensor); xres = raw-view x-slice + gamma*bv,
    fp16, host-prearranged.  Out DMAs split gpsimd/sync.
  - 36 back-to-back junk accum matmuls at t=0 ramp the HAM clock gate
    while input DMA is in flight (fewer warmups measured slower: the
    8/8 grant comes later and more of proj runs at half clock).
PSUM: warmup 1 + proj 6 (2 per tag) -> attention 4 S banks + 4 out.
Timeline on HW: ~6.7us fixed preamble; DMA+proj+ramp to ~24us;
4 x 27.4us attention (PE issue-bound, ~855ns/superstep); ~8us fixed
teardown (semaphore resets).  HW exec ~168us (baseline v4: 181us).
"""

import numpy as np

B, C, Dd, Hh, Ww = 2, 256, 8, 32, 32
N = Dd * Hh * Ww          # 8192
CQK = C // 8              # 32
NCORES = 8
QCHUNK = N // 4           # 2048 query rows per core
P = 128

WS = 64.0                 # host-side weight prescale (fp8 range)
A_F8 = 11.5416            # 2**3 / ln2  (fp8e4 schraudolph)
B_F8 = 55.7248            # 8 * (7 - 0.0344)
VPAD = 272                # 16B-aligned vT tile pitch (257 used)
NWARM = 36                # PE warmup matmuls (HAM clock ramp)


def build_graph(n=N, nq=QCHUNK):
    import concourse.tile as tile
    from concourse import bacc, mybir
    from concourse.bass import ds, ts

    f32 = mybir.dt.float32
    f16 = mybir.dt.float16
    bf16 = mybir.dt.bfloat16
    fp8 = mybir.dt.float8e4
    i8 = mybir.dt.int8
    AF = mybir.ActivationFunctionType
    ALU = mybir.AluOpType
    DR = mybir.MatmulPerfMode.DoubleRow

    n_t = n // 512            # 16 k-proj tiles
    m_tiles = n // P          # 64 key tiles (V proj)
    n_sc = nq // 512          # 4 query chunks
    n_ss2 = n // 256          # 32 supersteps (2 key tiles each) per sc
    nquart = 4                # key quarters (2048 keys each)

    nc = bacc.Bacc()
    xf8_d = nc.declare_dram_parameter("xf8", [P, 2, n], fp8, isOutput=False)
    xres_d = nc.declare_dram_parameter("xres", [P, nq // P, C], f16,
                                       isOutput=False)
    wq8r_d = nc.declare_dram_parameter("wq8r", [P, 2, P], fp8, isOutput=False)
    wk8_d = nc.declare_dram_parameter("wk8", [P, 2, CQK], fp8, isOutput=False)
    wv8_d = nc.declare_dram_parameter("wv8", [P, 2, C], fp8, isOutput=False)
    bqr_d = nc.declare_dram_parameter("bqr", [P, 1], f32, isOutput=False)
    bk_d = nc.declare_dram_parameter("bk", [CQK, 1], f32, isOutput=False)
    out_d = nc.declare_dram_parameter("out", [nq, C], f32, isOutput=True)

    with tile.TileContext(nc) as tc:
        with tc.tile_pool(name="singles", bufs=1) as singles, \
             tc.tile_pool(name="ostage", bufs=3) as ostage, \
             tc.tile_pool(name="ptp", bufs=6) as ptp:

            # ---- static SBUF ---------------------------------------------
            wq8r_s = singles.tile([P, 2, P], fp8)
            wk8_s = singles.tile([P, 2, CQK], fp8)
            wv8_s = singles.tile([P, 2, C], fp8)
            bqr_s = singles.tile([P, 1], f32)
            bk_s = singles.tile([P, 1], f32)
            junk = singles.tile([P, 640], bf16)
            xf8_s = singles.tile([P, 2, n], fp8)
            xres_s = singles.tile([P, nq // P, C], f16)
            k_stage = singles.tile([P, n_t, 512], bf16)
            # k_bd: per 128-key tile a block-diagonal [128, 128] lhsT --
            # rows 32a:32a+32 hold k[:, sub-block a] (dense PE stream;
            # tile_position variants measured slower: PE reconfig penalty
            # on every scores<->attn@v transition)
            k_bd = singles.tile([P, n // P, P], bf16)
            q_rep = singles.tile([P, n_sc, 512], bf16)
            vT8 = singles.tile([P, m_tiles, VPAD], fp8)

            # gpsimd queue: weight loads (contiguous host layouts)
            nc.gpsimd.dma_start(out=wq8r_s, in_=wq8r_d[:])
            nc.gpsimd.dma_start(out=wk8_s, in_=wk8_d[:])
            nc.gpsimd.dma_start(out=wv8_s, in_=wv8_d[:])
            nc.gpsimd.dma_start(out=bqr_s, in_=bqr_d[:])
            nc.gpsimd.dma_start(out=bk_s[0:CQK, :], in_=bk_d[:])

            # sync queue: xf8 quarters (host rotated: own quarter first)
            for qq in range(nquart):
                nc.sync.dma_start(out=xf8_s[:, :, ts(qq, n // 4)],
                                  in_=xf8_d[:, :, ts(qq, n // 4)])
            # xres after xf8 (needed only at the sc0 epilogue)
            for h in range(2):
                nc.sync.dma_start(
                    out=xres_s[:, ts(h, nq // P // 2), :],
                    in_=xres_d[:, ts(h, nq // P // 2), :])

            # vector queue first ops: memsets
            nc.vector.memset(junk, 0.25)
            nc.vector.memset(k_bd, 0.0)
            nc.vector.memset(vT8[:, :, C:C + 1], WS)

            # PE warmup: accumulate junk into one PSUM bank back-to-back so
            # the HAM clock gate ramps to 8/8 while input DMA is in flight
            with tc.tile_pool(name="warmp", bufs=1, space="PSUM") as wp:
                wps = wp.tile([P, 512], f32, tag="w", name="wps")
                for r in range(NWARM):
                    nc.tensor.matmul(wps, lhsT=junk[:, 0:P],
                                     rhs=junk[:, P:P + 512],
                                     start=(r == 0), stop=(r == NWARM - 1))

            # ---- projections (per key quarter, pipelined with DMA) -------
            ksr = k_stage[0:CQK, :, :].rearrange(
                "p t (f a kk) -> p (t f) a kk", a=4, kk=32)
            with tc.tile_pool(name="pp", bufs=2, space="PSUM") as pp:
                # q first: own chunk is always the first nq keys (host rot)
                for t in range(n_sc):
                    ps_q = pp.tile([P, 512], f32, tag="psq", name=f"ps_q{t}")
                    nc.tensor.matmul(ps_q, lhsT=wq8r_s,
                                     rhs=xf8_s[:, :, ts(t, 512)],
                                     start=True, stop=True, perf_mode=DR)
                    nc.scalar.activation(q_rep[:, t, :], ps_q,
                                         AF.Identity, bias=bqr_s,
                                         scale=1.0 / WS)
                for qq in range(nquart):
                    for tl in range(n_t // nquart):
                        t = qq * (n_t // nquart) + tl
                        ps_k = pp.tile([P, 512], f32, tag="psk",
                                       name=f"ps_k{t}")
                        nc.tensor.matmul(ps_k[0:CQK, :], lhsT=wk8_s,
                                         rhs=xf8_s[:, :, ts(t, 512)],
                                         start=True, stop=True, perf_mode=DR)
                        nc.scalar.activation(k_stage[0:CQK, t, :],
                                             ps_k[0:CQK, :],
                                             AF.Identity, bias=bk_s[0:CQK, :],
                                             scale=1.0 / WS)
                    # scatter this quarter's k into the block diagonals
                    for a in range(4):
                        nc.gpsimd.dma_start(
                            out=k_bd[ds(32 * a, 32), ds(16 * qq, 16),
                                     ds(32 * a, 32)],
                            in_=ksr[:, ds(16 * qq, 16), a, :])
                    # v: vT8[m, c] = fp8(WS * gamma * (wv @ xf))
                    for mpl in range(8):
                        mp = qq * 8 + mpl
                        ps_v = pp.tile([P, 2, C], f32, tag="psv",
                                       name=f"ps_v{mp}")
                        for h in range(2):
                            nc.tensor.matmul(ps_v[:, h, :],
                                             lhsT=xf8_s[:, :, ts(2 * mp + h, P)],
                                             rhs=wv8_s, start=True, stop=True,
                                             perf_mode=DR)
                        nc.vector.tensor_copy(vT8[:, ds(2 * mp, 2), 0:C],
                                              ps_v[:])

            # ---- attention ------------------------------------------------
            # warm the Exp activation table after the proj Identity passes
            warm = ostage.tile([P, 1], bf16, tag="warm", name="warm")
            nc.scalar.activation(warm[0:CQK, :], junk[0:CQK, 0:1], AF.Exp)
            outr = out_d[:].rearrange("(t p) c -> p t c", p=P)
            with tc.tile_pool(name="stp", bufs=4, space="PSUM") as stp, \
                 tc.tile_pool(name="op", bufs=1, space="PSUM") as op:
                for sc in range(n_sc):
                    out_ps = [op.tile([P, VPAD], f32, tag=f"ops{qt}",
                                      name=f"out_ps{sc}_{qt}")
                              for qt in range(4)]
                    pipe = []
                    for ss in range(n_ss2):
                        s0 = stp.tile([P, 512], f32, tag="s",
                                      name=f"s{sc}_{ss}a")
                        s1 = stp.tile([P, 512], f32, tag="s",
                                      name=f"s{sc}_{ss}b")
                        pt = ptp.tile([P, 2, 512], fp8, tag="pt",
                                      name=f"pt{sc}_{ss}")
                        nc.tensor.matmul(s0, lhsT=k_bd[:, 2 * ss, :],
                                         rhs=q_rep[:, sc, :],
                                         start=True, stop=True)
                        nc.tensor.matmul(s1, lhsT=k_bd[:, 2 * ss + 1, :],
                                         rhs=q_rep[:, sc, :],
                                         start=True, stop=True)
                        # split exp: ScalarE exact on tile 0, DVE
                        # schraudolph-int8 on tile 1 — both inside the
                        # superstep, so S banks free fast and PE never waits
                        nc.scalar.activation(pt[:, 0, :], s0, AF.Exp)
                        nc.vector.tensor_scalar(
                            out=pt[:, 1, :].bitcast(i8), in0=s1,
                            scalar1=A_F8, scalar2=B_F8,
                            op0=ALU.mult, op1=ALU.add)
                        pipe.append((ss, pt))
                        if len(pipe) > 1:
                            pss, ppt = pipe.pop(0)
                            for qt in range(4):
                                nc.tensor.matmul(
                                    out_ps[qt][:, 0:257],
                                    lhsT=ppt[:, :, ts(qt, P)],
                                    rhs=vT8[:, ds(2 * pss, 2), 0:257],
                                    start=(pss == 0), stop=False,
                                    perf_mode=DR)
                    for pss, ppt in pipe:
                        for qt in range(4):
                            nc.tensor.matmul(
                                out_ps[qt][:, 0:257],
                                lhsT=ppt[:, :, ts(qt, P)],
                                rhs=vT8[:, ds(2 * pss, 2), 0:257],
                                start=(pss == 0),
                                stop=(pss == n_ss2 - 1), perf_mode=DR)
                    # epilogue: out = psum[:, :C] / (WS*rowsum) * WS + xres
                    for qt in range(4):
                        rec = ostage.tile([P, 1], f32, tag="rec",
                                          name=f"rec{sc}_{qt}")
                        nc.vector.reciprocal(rec, out_ps[qt][:, 256:257])
                        ot = ostage.tile([P, C], f32, tag="ot",
                                         name=f"ot{sc}_{qt}")
                        nc.vector.scalar_tensor_tensor(
                            out=ot, in0=out_ps[qt][:, 0:C], scalar=rec,
                            in1=xres_s[:, 4 * sc + qt, :],
                            op0=ALU.mult, op1=ALU.add)
                        if qt % 2 == 0:
                            nc.gpsimd.dma_start(out=outr[:, 4 * sc + qt, :],
                                                in_=ot)
                        else:
                            nc.sync.dma_start(out=outr[:, 4 * sc + qt, :],
                                              in_=ot)
    nc.compile()
    return nc


_nc_cache = {}


def _get_graph(n=N, nq=QCHUNK):
    key = (n, nq)
    if key not in _nc_cache:
        _nc_cache[key] = build_graph(n, nq)
    return _nc_cache[key]


def _make_in_maps(x, wq, bq, wk, bk, wv, bv, gamma, n=N, nq=QCHUNK):
    import ml_dtypes
    f8 = ml_dtypes.float8_e4m3fn
    xf = np.ascontiguousarray(np.asarray(x, dtype=np.float32).reshape(B, C, n))
    g = float(np.asarray(gamma).reshape(-1)[0])
    # [B, 128, 2, n] with channel c = co*128 + p
    xf8 = np.ascontiguousarray(
        xf.astype(f8).reshape(B, 2, P, n).transpose(0, 2, 1, 3))

    def wlayout(w_t, cols):  # [C, cols] f32 -> [128, 2, cols] fp8
        return np.ascontiguousarray(
            w_t.astype(f8).reshape(2, P, cols).transpose(1, 0, 2))

    wqT = np.asarray(wq, dtype=np.float32).T * WS          # [C, 32]
    wq8r = wlayout(np.tile(wqT, (1, 4)), P)                # replicated x4
    wk8 = wlayout(np.asarray(wk, dtype=np.float32).T * WS, CQK)
    wv8 = wlayout(np.asarray(wv, dtype=np.float32).T * (WS * g), C)
    bqr = np.ascontiguousarray(
        np.tile(np.asarray(bq, dtype=np.float32), 4).reshape(P, 1))
    bk2 = np.asarray(bk, dtype=np.float32).reshape(CQK, 1)
    gbv = (g * np.asarray(bv, dtype=np.float32))[None, :]

    nchunks = n // nq
    in_maps = []
    for i in range(NCORES):
        b, c = divmod(i, nchunks)
        n0 = c * nq
        # rotate key quarters so this core's own quarter comes first
        order = [(c + j) % nchunks for j in range(nchunks)]
        perm = np.concatenate([np.arange(q * nq, (q + 1) * nq) for q in order])
        xf8_c = np.ascontiguousarray(xf8[b][:, :, perm])
        # residual for own queries: raw flat view of x (torch-faithful),
        # fp16, [128, 16, 256]
        xres = (xf[b].reshape(-1)[n0 * C:(n0 + nq) * C].reshape(nq, C)
                + gbv).astype(np.float16)
        xres = np.ascontiguousarray(
            xres.reshape(nq // P, P, C).transpose(1, 0, 2))
        in_maps.append({
            "xf8": xf8_c,
            "xres": xres,
            "wq8r": wq8r, "wk8": wk8, "wv8": wv8,
            "bqr": bqr, "bk": bk2,
        })
    return in_maps


def _assemble(results, n=N, nq=QCHUNK):
    nchunks = n // nq
    outs = []
    for b in range(B):
        buf = np.concatenate(
            [results[b * nchunks + c]["out"] for c in range(nchunks)], axis=0)
        outs.append(buf.reshape(C, Dd, Hh, Ww))
    return np.stack(outs).astype(np.float32)


def kernel(x, wq, bq, wk, bk, wv, bv, gamma):
    from concourse.bass_utils import run_bass_kernel_spmd
    nc = _get_graph()
    in_maps = _make_in_maps(x, wq, bq, wk, bk, wv, bv, gamma)
    res = run_bass_kernel_spmd(nc, in_maps, core_ids=list(range(NCORES)))
    return _assemble(res.results)


# revision 43
# speedup vs baseline: 1.0052x; 1.0052x over previous
"""Trainium2 Bass kernel for nn_AttentionBlock (B=2, C=256, D=8, H=32, W=32).

reference math:
    xf = x.reshape(B, C, N)                        # N = 8192
    q = wq @ xf + bq                               # (B, 32, N)
    k = wk @ xf + bk                               # (B, 32, N)
    v = wv @ xf + bv                               # (B, 256, N)
    attn = softmax(q^T k, axis=-1)                 # (B, N, N)
    out = attn @ v^T                               # (B, N, C) buffer
    result = gamma * out.reshape(B, C, d, h, w) + x

Sharding (8 cores): core i -> batch b = i//4, query-chunk c = i%4 of 2048
rows.  No collectives.

v6 design (v4 + split exp + host pre-layout + key rotation):
  - host pre-arranges every tensor into its exact SBUF layout so all
    input DMAs are contiguous (fast SWDGE descriptors, early start).
  - keys are rotated per-core on host so each core's own quarter is
    first: q = xf8[:, :, 0:2048] is a free SBUF view (no q DMA), and
    the projection pipeline starts right after the first 0.5MB lands.
  - wq is host-replicated x4 along output channels, so the q-projection
    matmul directly produces the 4x partition-replicated q_rep that the
    block-diagonal score matmuls need (no SBUF replicate DMAs).
  - projections: fp8 weights (prescaled x64), DoubleRow K=256; per
    quarter: 4 k-tiles + 16 v-tiles issued as DMA lands.  q/k bias
    descale on ScalarE (Identity), v-copies on DVE.
  - scores: S^T = k^T q as bf16 K=32 matmuls via the 4-quadrant
    block-diagonal k_bd (dense PE stream at 128 outputs/cycle).
    (tile_position row-group variants measured SLOWER: ~230ns PE
    reconfig penalty on every scores<->attn@v transition.)
  - softmax exp (|S| < 4.5, no max subtraction): each superstep's two
    512-col S banks split: bank0 -> ScalarE (ACTIVATE Exp -> fp8),
    bank1 -> DVE (Schraudolph round(a*S+b) as int8 bitcast fp8e4).
    Both run inside one 855ns superstep so the PE never waits on exp.
  - attn @ v: fp8 DoubleRow, pipe lag 1.  vT is 272-padded and carries
    a WS=64.0 column so PSUM accumulates WS*rowsum next to out.
  - epilogue: rec = 1/psum[:,256]; out = psum[:,0:256]*rec + xres
    (one DVE scalar_tensor_tensor); xres = raw-view x-slice + gamma*bv,
    fp16, host-prearranged.  Out DMAs split gpsimd/sync.
  - 36 back-to-back junk accum matmuls at t=0 ramp the HAM clock gate
    while input DMA is in flight (fewer warmups measured slower: the
    8/8 grant comes later and more of proj runs at half clock).
PSUM: warmup 1 + proj 6 (2 per tag) -> attention 4 S banks + 4 out.
Timeline on HW: ~6.7us fixed preamble; DMA+proj+ramp to ~24us;
4 x 27.4us attention (PE issue-bound, ~855ns/superstep); ~8us fixed
teardown (semaphore resets).  HW exec ~168us (baseline v4: 181us).
"""

import numpy as np

B, C, Dd, Hh, Ww = 2, 256, 8, 32, 32
N = Dd * Hh * Ww          # 8192
CQK = C // 8              # 32
NCORES = 8
QCHUNK = N // 4           # 2048 query rows per core
P = 128

WS = 64.0                 # host-side weight prescale (fp8 range)
A_F8 = 11.5416            # 2**3 / ln2  (fp8e4 schraudolph)
B_F8 = 55.7248            # 8 * (7 - 0.0344)
VPAD = 272                # 16B-aligned vT tile pitch (257 used)
NWARM = 36                # PE warmup matmuls (HAM clock ramp)


def build_graph(n=N, nq=QCHUNK):
    import concourse.tile as tile
    from concourse import bacc, mybir
    from concourse.bass import ds, ts

    f32 = mybir.dt.float32
    f16 = mybir.dt.float16
    bf16 = mybir.dt.bfloat16
    fp8 = mybir.dt.float8e4
    i8 = mybir.dt.int8
    AF = mybir.ActivationFunctionType
    ALU = mybir.AluOpType
    DR = mybir.MatmulPerfMode.DoubleRow

    n_t = n // 512            # 16 k-proj tiles
    m_tiles = n // P          # 64 key tiles (V proj)
    n_sc = nq // 512          # 4 query chunks
    n_ss2 = n // 256          # 32 supersteps (2 key tiles each) per sc
    nquart = 4                # key quarters (2048 keys each)

    nc = bacc.Bacc()
    xf8_d = nc.declare_dram_parameter("xf8", [P, 2, n], fp8, isOutput=False)
    xres_d = nc.declare_dram_parameter("xres", [P, nq // P, C], f16,
                                       isOutput=False)
    wq8r_d = nc.declare_dram_parameter("wq8r", [P, 2, P], fp8, isOutput=False)
    wk8_d = nc.declare_dram_parameter("wk8", [P, 2, CQK], fp8, isOutput=False)
    wv8_d = nc.declare_dram_parameter("wv8", [P, 2, C], fp8, isOutput=False)
    bqr_d = nc.declare_dram_parameter("bqr", [P, 1], f32, isOutput=False)
    bk_d = nc.declare_dram_parameter("bk", [CQK, 1], f32, isOutput=False)
    out_d = nc.declare_dram_parameter("out", [nq, C], f32, isOutput=True)

    with tile.TileContext(nc) as tc:
        with tc.tile_pool(name="singles", bufs=1) as singles, \
             tc.tile_pool(name="ostage", bufs=3) as ostage, \
             tc.tile_pool(name="ptp", bufs=6) as ptp:

            # ---- static SBUF ---------------------------------------------
            wq8r_s = singles.tile([P, 2, P], fp8)
            wk8_s = singles.tile([P, 2, CQK], fp8)
            wv8_s = singles.tile([P, 2, C], fp8)
            bqr_s = singles.tile([P, 1], f32)
            bk_s = singles.tile([P, 1], f32)
            junk = singles.tile([P, 640], bf16)
            xf8_s = singles.tile([P, 2, n], fp8)
            xres_s = singles.tile([P, nq // P, C], f16)
            k_stage = singles.tile([P, n_t, 512], bf16)
            # k_bd: per 128-key tile a block-diagonal [128, 128] lhsT --
            # rows 32a:32a+32 hold k[:, sub-block a] (dense PE stream;
            # tile_position variants measured slower: PE reconfig penalty
            # on every scores<->attn@v transition)
            k_bd = singles.tile([P, n // P, P], bf16)
            q_rep = singles.tile([P, n_sc, 512], bf16)
            vT8 = singles.tile([P, m_tiles, VPAD], fp8)

            # gpsimd queue: weight loads (contiguous host layouts)
            nc.gpsimd.dma_start(out=wq8r_s, in_=wq8r_d[:])
            nc.gpsimd.dma_start(out=wk8_s, in_=wk8_d[:])
            nc.gpsimd.dma_start(out=wv8_s, in_=wv8_d[:])
            nc.gpsimd.dma_start(out=bqr_s, in_=bqr_d[:])
            nc.gpsimd.dma_start(out=bk_s[0:CQK, :], in_=bk_d[:])

            # sync queue: xf8 in 8 half-quarter chunks (host rotated: own
            # quarter first) — desc-gen pipelines with transfers, so each
            # 2-tile chunk unblocks its k/v projections ~0.7us sooner
            for hh in range(2 * nquart):
                nc.sync.dma_start(out=xf8_s[:, :, ts(hh, n // 8)],
                                  in_=xf8_d[:, :, ts(hh, n // 8)])
            # xres after xf8 (needed only at the sc0 epilogue)
            for h in range(2):
                nc.sync.dma_start(
                    out=xres_s[:, ts(h, nq // P // 2), :],
                    in_=xres_d[:, ts(h, nq // P // 2), :])

            # vector queue first ops: memsets
            nc.vector.memset(junk, 0.25)
            nc.vector.memset(k_bd, 0.0)
            nc.vector.memset(vT8[:, :, C:C + 1], WS)

            # PE warmup: accumulate junk into one PSUM bank back-to-back so
            # the HAM clock gate ramps to 8/8 while input DMA is in flight
            with tc.tile_pool(name="warmp", bufs=1, space="PSUM") as wp:
                wps = wp.tile([P, 512], f32, tag="w", name="wps")
                for r in range(NWARM):
                    nc.tensor.matmul(wps, lhsT=junk[:, 0:P],
                                     rhs=junk[:, P:P + 512],
                                     start=(r == 0), stop=(r == NWARM - 1))

            # ---- projections (per key quarter, pipelined with DMA) -------
            ksr = k_stage[0:CQK, :, :].rearrange(
                "p t (f a kk) -> p (t f) a kk", a=4, kk=32)
            with tc.tile_pool(name="pp", bufs=2, space="PSUM") as pp:
                # q first: own chunk is always the first nq keys (host rot)
                for t in range(n_sc):
                    ps_q = pp.tile([P, 512], f32, tag="psq", name=f"ps_q{t}")
                    nc.tensor.matmul(ps_q, lhsT=wq8r_s,
                                     rhs=xf8_s[:, :, ts(t, 512)],
                                     start=True, stop=True, perf_mode=DR)
                    nc.scalar.activation(q_rep[:, t, :], ps_q,
                                         AF.Identity, bias=bqr_s,
                                         scale=1.0 / WS)
                for qq in range(nquart):
                    for tl in range(n_t // nquart):
                        t = qq * (n_t // nquart) + tl
                        ps_k = pp.tile([P, 512], f32, tag="psk",
                                       name=f"ps_k{t}")
                        nc.tensor.matmul(ps_k[0:CQK, :], lhsT=wk8_s,
                                         rhs=xf8_s[:, :, ts(t, 512)],
                                         start=True, stop=True, perf_mode=DR)
                        nc.scalar.activation(k_stage[0:CQK, t, :],
                                             ps_k[0:CQK, :],
                                             AF.Identity, bias=bk_s[0:CQK, :],
                                             scale=1.0 / WS)
                    # scatter this quarter's k into the block diagonals
                    for a in range(4):
                        nc.gpsimd.dma_start(
                            out=k_bd[ds(32 * a, 32), ds(16 * qq, 16),
                                     ds(32 * a, 32)],
                            in_=ksr[:, ds(16 * qq, 16), a, :])
                    # v: vT8[m, c] = fp8(WS * gamma * (wv @ xf))
                    for mpl in range(8):
                        mp = qq * 8 + mpl
                        ps_v = pp.tile([P, 2, C], f32, tag="psv",
                                       name=f"ps_v{mp}")
                        for h in range(2):
                            nc.tensor.matmul(ps_v[:, h, :],
                                             lhsT=xf8_s[:, :, ts(2 * mp + h, P)],
                                             rhs=wv8_s, start=True, stop=True,
                                             perf_mode=DR)
                        nc.vector.tensor_copy(vT8[:, ds(2 * mp, 2), 0:C],
                                              ps_v[:])

            # ---- attention ------------------------------------------------
            # warm the Exp activation table after the proj Identity passes
            warm = ostage.tile([P, 1], bf16, tag="warm", name="warm")
            nc.scalar.activation(warm[0:CQK, :], junk[0:CQK, 0:1], AF.Exp)
            outr = out_d[:].rearrange("(t p) c -> p t c", p=P)
            with tc.tile_pool(name="stp", bufs=4, space="PSUM") as stp, \
                 tc.tile_pool(name="op", bufs=1, space="PSUM") as op:
                for sc in range(n_sc):
                    out_ps = [op.tile([P, VPAD], f32, tag=f"ops{qt}",
                                      name=f"out_ps{sc}_{qt}")
                              for qt in range(4)]
                    pipe = []
                    for ss in range(n_ss2):
                        s0 = stp.tile([P, 512], f32, tag="s",
                                      name=f"s{sc}_{ss}a")
                        s1 = stp.tile([P, 512], f32, tag="s",
                                      name=f"s{sc}_{ss}b")
                        pt = ptp.tile([P, 2, 512], fp8, tag="pt",
                                      name=f"pt{sc}_{ss}")
                        nc.tensor.matmul(s0, lhsT=k_bd[:, 2 * ss, :],
                                         rhs=q_rep[:, sc, :],
                                         start=True, stop=True)
                        nc.tensor.matmul(s1, lhsT=k_bd[:, 2 * ss + 1, :],
                                         rhs=q_rep[:, sc, :],
                                         start=True, stop=True)
                        # split exp: ScalarE exact on tile 0, DVE
                        # schraudolph-int8 on tile 1 — both inside the
                        # superstep, so S banks free fast and PE never waits
                        nc.scalar.activation(pt[:, 0, :], s0, AF.Exp)
                        nc.vector.tensor_scalar(
                            out=pt[:, 1, :].bitcast(i8), in0=s1,
                            scalar1=A_F8, scalar2=B_F8,
                            op0=ALU.mult, op1=ALU.add)
                        pipe.append((ss, pt))
                        if len(pipe) > 1:
                            pss, ppt = pipe.pop(0)
                            for qt in range(4):
                                nc.tensor.matmul(
                                    out_ps[qt][:, 0:257],
                                    lhsT=ppt[:, :, ts(qt, P)],
                                    rhs=vT8[:, ds(2 * pss, 2), 0:257],
                                    start=(pss == 0), stop=False,
                                    perf_mode=DR)
                    for pss, ppt in pipe:
                        for qt in range(4):
                            nc.tensor.matmul(
                                out_ps[qt][:, 0:257],
                                lhsT=ppt[:, :, ts(qt, P)],
                                rhs=vT8[:, ds(2 * pss, 2), 0:257],
                                start=(pss == 0),
                                stop=(pss == n_ss2 - 1), perf_mode=DR)
                    # epilogue: out = psum[:, :C] / (WS*rowsum) * WS + xres
                    for qt in range(4):
                        rec = ostage.tile([P, 1], f32, tag="rec",
                                          name=f"rec{sc}_{qt}")
                        nc.vector.reciprocal(rec, out_ps[qt][:, 256:257])
                        ot = ostage.tile([P, C], f32, tag="ot",
                                         name=f"ot{sc}_{qt}")
                        nc.vector.scalar_tensor_tensor(
                            out=ot, in0=out_ps[qt][:, 0:C], scalar=rec,
                            in1=xres_s[:, 4 * sc + qt, :],
                            op0=ALU.mult, op1=ALU.add)
                        if qt % 2 == 0:
                            nc.gpsimd.dma_start(out=outr[:, 4 * sc + qt, :],
                                                in_=ot)
                        else:
                            nc.sync.dma_start(out=outr[:, 4 * sc + qt, :],
                                              in_=ot)
    nc.compile()
    return nc


_nc_cache = {}


def _get_graph(n=N, nq=QCHUNK):
    key = (n, nq)
    if key not in _nc_cache:
        _nc_cache[key] = build_graph(n, nq)
    return _nc_cache[key]


def _make_in_maps(x, wq, bq, wk, bk, wv, bv, gamma, n=N, nq=QCHUNK):
    import ml_dtypes
    f8 = ml_dtypes.float8_e4m3fn
    xf = np.ascontiguousarray(np.asarray(x, dtype=np.float32).reshape(B, C, n))
    g = float(np.asarray(gamma).reshape(-1)[0])
    # [B, 128, 2, n] with channel c = co*128 + p
    xf8 = np.ascontiguousarray(
        xf.astype(f8).reshape(B, 2, P, n).transpose(0, 2, 1, 3))

    def wlayout(w_t, cols):  # [C, cols] f32 -> [128, 2, cols] fp8
        return np.ascontiguousarray(
            w_t.astype(f8).reshape(2, P, cols).transpose(1, 0, 2))

    wqT = np.asarray(wq, dtype=np.float32).T * WS          # [C, 32]
    wq8r = wlayout(np.tile(wqT, (1, 4)), P)                # replicated x4
    wk8 = wlayout(np.asarray(wk, dtype=np.float32).T * WS, CQK)
    wv8 = wlayout(np.asarray(wv, dtype=np.float32).T * (WS * g), C)
    bqr = np.ascontiguousarray(
        np.tile(np.asarray(bq, dtype=np.float32), 4).reshape(P, 1))
    bk2 = np.asarray(bk, dtype=np.float32).reshape(CQK, 1)
    gbv = (g * np.asarray(bv, dtype=np.float32))[None, :]

    nchunks = n // nq
    in_maps = []
    for i in range(NCORES):
        b, c = divmod(i, nchunks)
        n0 = c * nq
        # rotate key quarters so this core's own quarter comes first
        order = [(c + j) % nchunks for j in range(nchunks)]
        perm = np.concatenate([np.arange(q * nq, (q + 1) * nq) for q in order])
        xf8_c = np.ascontiguousarray(xf8[b][:, :, perm])
        # residual for own queries: raw flat view of x (torch-faithful),
        # fp16, [128, 16, 256]
        xres = (xf[b].reshape(-1)[n0 * C:(n0 + nq) * C].reshape(nq, C)
                + gbv).astype(np.float16)
        xres = np.ascontiguousarray(
            xres.reshape(nq // P, P, C).transpose(1, 0, 2))
        in_maps.append({
            "xf8": xf8_c,
            "xres": xres,
            "wq8r": wq8r, "wk8": wk8, "wv8": wv8,
            "bqr": bqr, "bk": bk2,
        })
    return in_maps


def _assemble(results, n=N, nq=QCHUNK):
    nchunks = n // nq
    outs = []
    for b in range(B):
        buf = np.concatenate(
            [results[b * nchunks + c]["out"] for c in range(nchunks)], axis=0)
        outs.append(buf.reshape(C, Dd, Hh, Ww))
    return np.stack(outs).astype(np.float32)


def kernel(x, wq, bq, wk, bk, wv, bv, gamma):
    from concourse.bass_utils import run_bass_kernel_spmd
    nc = _get_graph()
    in_maps = _make_in_maps(x, wq, bq, wk, bk, wv, bv, gamma)
    res = run_bass_kernel_spmd(nc, in_maps, core_ids=list(range(NCORES)))
    return _assemble(res.results)
